# revision 1
# baseline (speedup 1.0000x reference)
import os

os.environ.setdefault("MYCRO_LOCAL_CACHE", "1")

import sys

if "/opt/trn_rl_repo" not in sys.path:
    sys.path.insert(0, "/opt/trn_rl_repo")

import numpy as np
import ml_dtypes

import concourse.bass as bass
import concourse.bacc as bacc
import concourse.tile as tile
from concourse.tile_rust import add_dep_helper
from concourse import mybir
from concourse.bass_utils import run_bass_kernel_spmd

BF16 = ml_dtypes.bfloat16

B, N, DIN, DOUT, E, CHEB_K = 32, 500, 64, 64, 16, 3
NCORES = 8
BPC = B // NCORES  # batches per core
NT = 4  # node tiles per batch
NTS = [128, 128, 128, 116]  # nodes per tile
NPAD = 512  # padded node-column block

f32 = mybir.dt.float32
f32r = mybir.dt.float32r
bf16 = mybir.dt.bfloat16

_CACHE = {}

# xmisc free-layout offsets
XA_OFF = 0  # [128, 4, 65]  xa chunks ([x | 1])
S_OFF = 260  # [128, 4, 16]  station_emb chunks
XM_W = 260 + 64


def _build_program():
    """Build the per-core SPMD program (same for all cores).

    Math (per batch; P = exp(relu(emb @ emb.T)) is symmetric):
      u1T[d,n]  = sum_m xa[m,d] P[m,n]          (row 64 = softmax denominators)
      x_g1T     = u1T * rinv_bcast              -> gA rows 64:127
      x_g1n     = transpose(x_g1T)              (PE, via identity rhs)
      u2T[d,n]  = sum_m x_g1n[m,d] P[m,n]
      x_g2'T    = u2T * rinv_bcast - 0.5 xT     -> gB rows 0:63 (2x in W2)
      y[n,o*16+e] = [xT;x_g1T].T @ W1 + [x_g2'T;1].T @ W2  (bias via ones row)
      out[n,o]  = sum_e y * s                   (bcast mul + bf16 tree adds)
    """
    if "nc" in _CACHE:
        return _CACHE["nc"]

    nc = bacc.Bacc(target_bir_lowering=False, trn_type="TRN2", debug=False)
    AF = mybir.ActivationFunctionType
    ALU = mybir.AluOpType

    # DRAM parameters (per-core shard)
    embT_d = nc.dram_tensor("embT", [16, BPC * NPAD], f32r, kind="ExternalInput")
    xm_d = nc.dram_tensor("xmisc", [BPC, 128, XM_W], bf16, kind="ExternalInput")
    xT_d = nc.dram_tensor("xT", [BPC, 64, NPAD], bf16, kind="ExternalInput")
    xhT_d = nc.dram_tensor("xhT", [BPC, 64, NPAD], bf16, kind="ExternalInput")
    ones_d = nc.dram_tensor("onesrow", [1, NPAD], bf16, kind="ExternalInput")
    W1_d = nc.dram_tensor("W1", [128, 1024], bf16, kind="ExternalInput")
    W2_d = nc.dram_tensor("W2", [65, 1024], bf16, kind="ExternalInput")
    id_d = nc.dram_tensor("ident", [128, 64], bf16, kind="ExternalInput")
    out_d = nc.dram_tensor("out", [BPC, 128, 256], bf16, kind="ExternalOutput")
    rdram_d = nc.dram_tensor(
        "rscratch",
        [BPC, 500],
        f32,
        kind="ExternalOutput"
        if os.environ.get("KERNEL_DEBUG", "0") == "1"
        else "Internal",
    )
    dbg = os.environ.get("KERNEL_DEBUG", "0") == "1"
    if dbg:
        dbgP_d = nc.dram_tensor("dbgP", [128, 2048], bf16, kind="ExternalOutput")
        dbgJu1_d = nc.dram_tensor("dbgJu1", [65, 500], f32, kind="ExternalOutput")
        dbgrB_d = nc.dram_tensor("dbgrB", [64, 500], f32, kind="ExternalOutput")
        dbggA_d = nc.dram_tensor("dbggA", [128, NPAD], bf16, kind="ExternalOutput")
        dbgx1n_d = nc.dram_tensor("dbgx1n", [128, 256], bf16, kind="ExternalOutput")
        dbgJu2_d = nc.dram_tensor("dbgJu2", [64, 500], f32, kind="ExternalOutput")
        dbggB_d = nc.dram_tensor("dbggB", [65, NPAD], bf16, kind="ExternalOutput")
        dbgzz_d = nc.dram_tensor("dbgzz", [128, 1152], bf16, kind="ExternalOutput")

    with tile.TileContext(nc) as tc:
        with (
            tc.tile_pool(name="cpool", bufs=1) as cpool,
            tc.tile_pool(name="spool", bufs=4) as spool,
            tc.tile_pool(name="pS", bufs=2, space="PSUM") as pS,
            tc.tile_pool(name="pW", bufs=2, space="PSUM") as pW,
            tc.tile_pool(name="pY", bufs=2, space="PSUM") as pY,
        ):
            W1s = cpool.tile([128, 1024], bf16, name="W1s", tag="W1s")
            nc.sync.dma_start(W1s, W1_d.ap())
            W2s = cpool.tile([65, 1024], bf16, name="W2s", tag="W2s")
            nc.sync.dma_start(W2s, W2_d.ap())
            ids = cpool.tile([128, 64], bf16, name="ids", tag="ids")
            nc.sync.dma_start(ids, id_d.ap())
            embTall = cpool.tile([16, BPC * NPAD], f32r, name="embTall", tag="embT")
            nc.sync.dma_start(embTall, embT_d.ap())

            reps = int(os.environ.get("KERNEL_REPS", "1"))
            import contextlib

            loop_cm = tc.For_i(0, reps, 1) if reps > 1 else contextlib.nullcontext()
            with loop_cm:
                prev_u2_first = None
                for b in range(BPC):
                    # ---- input DMAs ----
                    embT_s = embTall[:, NPAD * b : NPAD * (b + 1)]
                    xm_s = spool.tile([128, XM_W], bf16, name="xm_s", tag="xm")
                    nc.sync.dma_start(xm_s, xm_d.ap()[b])
                    gA = spool.tile([128, NPAD], bf16, name="gA", tag="gA")
                    nc.sync.dma_start(gA[0:64, :], xT_d.ap()[b])
                    xhT_s = spool.tile([64, NPAD], bf16, name="xhT_s", tag="xhT")
                    nc.scalar.dma_start(xhT_s, xhT_d.ap()[b])
                    gB = spool.tile([65, NPAD], bf16, name="gB", tag="gB")
                    nc.scalar.dma_start(gB[64:65, :], ones_d.ap())

                    # ---- scores S = embT.T @ embT (f32r, full rate at N=512),
                    #      then P = max(exp(S), 1)  (== exp(relu(S))) ----
                    P = spool.tile([128, 2048], bf16, name="P", tag="P")
                    for h in range(2):
                        Sp = pS.tile([128, 1024], f32, name="Sp", tag="S")
                        for tl in range(2):
                            t = 2 * h + tl
                            nt = NTS[t]
                            mm = nc.tensor.matmul(
                                Sp[0:nt, 512 * tl : 512 * tl + 512],
                                embT_s[:, 128 * t : 128 * t + nt],
                                embT_s,
                                start=True,
                                stop=True,
                            )
                            if prev_u2_first is not None:
                                # Order batch-b scores after batch-(b-1)'s u2 on
                                # PE so the psum-slot ACT wait is vector-clock
                                # redundant (walrus takes few sync waits per op).
                                add_dep_helper(
                                    mm.ins,
                                    prev_u2_first.ins,
                                    sync=False,
                                    reason="scores after prev u2",
                                )
                        if h == 0:
                            nc.scalar.activation(P[:, 0:1024], Sp, AF.Exp)
                            nc.vector.tensor_scalar_max(
                                P[:, 0:1024], P[:, 0:1024], 1.0
                            )
                        else:
                            nc.scalar.activation(
                                P[:, 1024:1536], Sp[:, 0:512], AF.Exp
                            )
                            nc.scalar.activation(
                                P[0:116, 1536:2048], Sp[0:116, 512:1024], AF.Exp
                            )
                            nc.vector.tensor_scalar_max(
                                P[:, 1024:1536], P[:, 1024:1536], 1.0
                            )
                            nc.vector.tensor_scalar_max(
                                P[0:116, 1536:2048], P[0:116, 1536:2048], 1.0
                            )

                    if dbg and b == 0:
                        nc.sync.dma_start(dbgP_d.ap(), P)

                    # ---- u1T = xa.T @ P  [65, 500]; row 64 = row sums of P ----
                    Ju1 = pW.tile([65, 500], f32, name="Ju1", tag="w")
                    for c in range(NT):
                        kc = NTS[c]
                        nc.tensor.matmul(
                            Ju1,
                            xm_s[0:kc, XA_OFF + 65 * c : XA_OFF + 65 * c + 65],
                            P[0:kc, 512 * c : 512 * c + 500],
                            start=(c == 0),
                            stop=(c == 3),
                        )

                    # rinv row + broadcast to 64 partitions
                    rT = spool.tile([1, 500], f32, name="rT", tag="rT")
                    # NB: reciprocal_approx_fast NaNs on HW for large inputs
                    # (row sums reach ~1e13); use the exact iterative divide.
                    nc.vector.reciprocal(rT, Ju1[64:65, 0:500])
                    rB = spool.tile([64, 500], f32, name="rB", tag="rB")
                    if os.environ.get("NO_GPSIMD", "0") == "1":
                        nc.sync.dma_start(rdram_d.ap()[b], rT)
                        nc.sync.dma_start(
                            rB,
                            rdram_d.ap()[b]
                            .rearrange("(x n) -> x n", x=1)
                            .broadcast_to((64, 500)),
                        )
                    else:
                        nc.gpsimd.partition_broadcast(rB, rT)

                    if dbg and b == 0:
                        nc.sync.dma_start(dbgrB_d.ap(), rB)

                    # ---- x_g1T = u1T * rinv  -> gA rows 64:127 ----
                    nc.vector.tensor_tensor(
                        gA[64:128, 0:500], Ju1[0:64, 0:500], rB, op=ALU.mult
                    )

                    # ---- x_g1n = transpose(x_g1T) via identity rhs ----
                    Jg1 = pW.tile([128, 256], f32, name="Jg1", tag="w")
                    for t in range(NT):
                        nt = NTS[t]
                        nc.tensor.matmul(
                            Jg1[0:nt, 64 * t : 64 * t + 64],
                            gA[64:128, 128 * t : 128 * t + nt],
                            ids[64:128, :],
                            start=True,
                            stop=True,
                        )
                    xg1n = spool.tile([128, 256], bf16, name="xg1n", tag="xg1n")
                    nc.vector.tensor_copy(xg1n[:, 0:192], Jg1[:, 0:192])
                    nc.vector.tensor_copy(xg1n[0:116, 192:256], Jg1[0:116, 192:256])

                    if dbg and b == 0:
                        nc.sync.dma_start(dbggA_d.ap(), gA)
                        nc.sync.dma_start(dbgx1n_d.ap()[:, 0:192], xg1n[:, 0:192])
                        nc.sync.dma_start(
                            dbgx1n_d.ap()[0:116, 192:256], xg1n[0:116, 192:256]
                        )

                    # ---- u2T = x_g1n.T @ P  [64, 500] ----
                    Ju2 = pW.tile([64, 500], f32, name="Ju2", tag="w")
                    prev_u2_first = None
                    for c in range(NT):
                        kc = NTS[c]
                        mm = nc.tensor.matmul(
                            Ju2,
                            xg1n[0:kc, 64 * c : 64 * c + 64],
                            P[0:kc, 512 * c : 512 * c + 500],
                            start=(c == 0),
                            stop=(c == 3),
                        )
                        if prev_u2_first is None:
                            prev_u2_first = mm

                    # ---- x_g2'T = u2T * rinv - 0.5 xT  -> gB rows 0:63 ----
                    xg2t = spool.tile([64, 512], bf16, name="xg2t", tag="xg2t")
                    nc.vector.tensor_tensor(
                        xg2t[:, 0:500], Ju2[:, 0:500], rB, op=ALU.mult
                    )
                    nc.vector.tensor_tensor(
                        gB[0:64, 0:500],
                        xg2t[:, 0:500],
                        xhT_s[:, 0:500],
                        op=ALU.subtract,
                    )

                    if dbg and b == 0:
                        nc.sync.dma_start(dbggB_d.ap(), gB)

                    # ---- y = gT.T @ Waug ; z = y * s ; out = sum_e z ----
                    out_s = spool.tile([128, 256], bf16, name="out_s", tag="out")
                    last = b == BPC - 1
                    for t in range(NT):
                        nt = NTS[t]
                        # o-stride 18 (2 pad cols per o-group): keeps the (o, e)
                        # dims un-mergeable and every o-group 4B-aligned so the
                        # bf16 tree adds can hit the DVE 2x mode.
                        zz = spool.tile([128, 1152], bf16, name="zz", tag="zz")
                        evac = t < 2 or last  # ACT evacuates psum for these tiles
                        for fc in range(2):
                            yp = pY.tile([128, 512], f32, name="yp", tag="y")
                            nc.tensor.matmul(
                                yp[0:nt, :],
                                gA[:, 128 * t : 128 * t + nt],
                                W1s[:, 512 * fc : 512 * fc + 512],
                                start=True,
                                stop=False,
                            )
                            nc.tensor.matmul(
                                yp[0:nt, :],
                                gB[:, 128 * t : 128 * t + nt],
                                W2s[:, 512 * fc : 512 * fc + 512],
                                start=False,
                                stop=True,
                            )
                            z3 = zz[0:nt, 576 * fc : 576 * fc + 576].rearrange(
                                "p (o e) -> p o e", e=18
                            )[:, :, 0:16]
                            y3 = yp[0:nt, :].rearrange("p (o e) -> p o e", e=16)
                            if evac:
                                nc.scalar.copy(z3, y3)
                            else:
                                s3 = (
                                    xm_s[0:nt, S_OFF + 16 * t : S_OFF + 16 * t + 16]
                                    .rearrange("p (x e) -> p x e", x=1)
                                    .broadcast_to((nt, 32, 16))
                                )
                                nc.vector.tensor_tensor(z3, y3, s3, op=ALU.mult)
                        zt = zz[0:nt, :].rearrange("p (o e) -> p o e", e=18)
                        if evac:
                            # multiply the evacuated bf16 y by s in place
                            sfull = (
                                xm_s[0:nt, S_OFF + 16 * t : S_OFF + 16 * t + 16]
                                .rearrange("p (x e) -> p x e", x=1)
                                .broadcast_to((nt, 64, 16))
                            )
                            eng = (
                            nc.gpsimd
                            if (t == 0 and os.environ.get("NO_GPSIMD", "0") != "1")
                            else nc.vector
                        )
                            eng.tensor_tensor(
                                zt[:, :, 0:16], zt[:, :, 0:16], sfull, op=ALU.mult
                            )
                        if dbg and b == 0 and t == 0:
                            nc.sync.dma_start(dbgzz_d.ap(), zz)

                        # binary-tree e-reduction (bf16, DVE 2x-eligible)
                        teng = (
                        nc.gpsimd
                        if (
                            last
                            and t == 0
                            and os.environ.get("NO_GPSIMD", "0") != "1"
                        )
                        else nc.vector
                    )
                        teng.tensor_tensor(
                            zt[:, :, 0:8], zt[:, :, 0:8], zt[:, :, 8:16], op=ALU.add
                        )
                        teng.tensor_tensor(
                            zt[:, :, 0:4], zt[:, :, 0:4], zt[:, :, 4:8], op=ALU.add
                        )
                        teng.tensor_tensor(
                            zt[:, :, 0:2], zt[:, :, 0:2], zt[:, :, 2:4], op=ALU.add
                        )
                        teng.tensor_tensor(
                            out_s[0:nt, 64 * t : 64 * t + 64],
                            zt[:, :, 0:1],
                            zt[:, :, 1:2],
                            op=ALU.add,
                        )

                    nc.scalar.dma_start(out_d.ap()[b][0:128, 0:192], out_s[:, 0:192])
                    nc.scalar.dma_start(
                        out_d.ap()[b][0:116, 192:256], out_s[0:116, 192:256]
                    )

    nc.compile()
    _CACHE["nc"] = nc
    return nc


def _prep_inputs(x, all_emb, station_emb, weights_pool, bias_pool):
    """Host-side layout prep. Returns in_maps (one dict per core)."""
    x = np.asarray(x, np.float32)
    all_emb = np.asarray(all_emb, np.float32)
    station_emb = np.asarray(station_emb, np.float32)
    weights_pool = np.asarray(weights_pool, np.float32)
    bias_pool = np.asarray(bias_pool, np.float32)

    # W'[k*64+i, o*16+e] = weights_pool[e, k, i, o]
    Wp = np.transpose(weights_pool, (1, 2, 3, 0))  # [k, i, o, e]
    W1 = Wp[0:2].reshape(128, 1024).astype(BF16)  # k=0 (x), k=1 (x_g1)
    W2 = np.empty((65, 1024), np.float32)
    W2[0:64] = 2.0 * Wp[2].reshape(64, 1024)  # k=2 (x_g2'), doubled
    W2[64] = np.transpose(bias_pool, (1, 0)).reshape(1024)  # ones row -> bias
    W2 = W2.astype(BF16)
    ident = np.tile(np.eye(64, dtype=np.float32), (2, 1)).astype(BF16)
    onesrow = np.ones((1, NPAD), np.float32).astype(BF16)

    node_valid = (
        np.arange(4)[:, None] * 128 + np.arange(128)[None, :]
    ) < N  # [4, 128]

    in_maps = []
    for core in range(NCORES):
        b0 = core * BPC
        xb = x[b0 : b0 + BPC]  # [4, 500, 64]
        eb = all_emb[b0 : b0 + BPC]
        sb = station_emb[b0 : b0 + BPC]

        embT = np.zeros((BPC, 16, NPAD), np.float32)
        embT[:, :, 0:N] = np.transpose(eb, (0, 2, 1))
        embT = np.transpose(embT, (1, 0, 2)).reshape(16, BPC * NPAD)

        xpad = np.zeros((BPC, NPAD, DIN), np.float32)
        xpad[:, 0:N] = xb
        xch = np.transpose(xpad.reshape(BPC, 4, 128, DIN), (0, 2, 1, 3))

        xm = np.zeros((BPC, 128, XM_W), np.float32)
        xa = np.concatenate(
            [
                xch,
                np.broadcast_to(
                    np.transpose(node_valid, (1, 0))[None, :, :, None],
                    (BPC, 128, 4, 1),
                ).astype(np.float32),
            ],
            axis=3,
        )  # [BPC, 128, 4, 65]
        xm[:, :, XA_OFF : XA_OFF + 260] = xa.reshape(BPC, 128, 260)
        spad = np.zeros((BPC, NPAD, E), np.float32)
        spad[:, 0:N] = sb
        xm[:, :, S_OFF : S_OFF + 64] = np.transpose(
            spad.reshape(BPC, 4, 128, E), (0, 2, 1, 3)
        ).reshape(BPC, 128, 64)

        xT = np.zeros((BPC, 64, NPAD), np.float32)
        xT[:, :, 0:N] = np.transpose(xb, (0, 2, 1))

        in_maps.append(
            {
                "embT": embT,
                "xmisc": xm.astype(BF16),
                "xT": xT.astype(BF16),
                "xhT": (0.5 * xT).astype(BF16),
                "onesrow": onesrow,
                "W1": W1,
                "W2": W2,
                "ident": ident,
            }
        )
    return in_maps


def _gather(results):
    """results: list of per-core dicts with 'out' [BPC, 128, 256] bf16."""
    out = np.zeros((B, N, DOUT), np.float32)
    for core in range(NCORES):
        r = np.asarray(results[core]["out"], dtype=np.float32)  # [4,128,256]
        r = r.reshape(BPC, 128, 4, 64)
        for t in range(NT):
            nt = NTS[t]
            out[core * BPC : (core + 1) * BPC, 128 * t : 128 * t + nt, :] = r[
                :, 0:nt, t, :
            ]
    return out


def kernel(_trace=False, _trace_kwargs=None, **inputs):
    nc = _build_program()
    in_maps = _prep_inputs(**inputs)
    res = run_bass_kernel_spmd(
        nc,
        in_maps,
        core_ids=list(range(NCORES)),
        trace=_trace,
        **(_trace_kwargs or {}),
    )
    _CACHE["last_result"] = res
    return _gather(res.results)

